# revision 21
# baseline (speedup 1.0000x reference)
"""Multi-head attention block (pre-LN, residual) on 8 Trainium2 NeuronCores.

Sharding: (batch x head-group) grid. Core c handles batch b = c//2 and head
group g = c%2 (8 of 16 heads). Per core: LN(x_b) -> per-head QKV projections
-> softmax attention (no max-subtraction; scores are O(10)) -> out-projection
against the local 512-wide slice of Wo, + 0.5*(x+bo) residual. Host sums the
two partial outputs per batch (the pair all-reduce) and stacks batches.

LayerNorm gamma/beta are folded into the QKV weights/biases on the host
(exact: projections are linear in xn). The 1/sqrt(E) score scale is folded
into Wq. Matmul operands are bf16 with fp32 PSUM accumulation; LN statistics,
softmax denominators and the residual path stay fp32.
"""

import numpy as np
import ml_dtypes

import concourse.bass as bass
import concourse.mybir as mybir
import concourse.tile as tile
from concourse import bacc
from concourse import bass_utils
from concourse.bass import ts
from concourse.masks import make_identity

BF_NP = ml_dtypes.bfloat16

B, S, D = 4, 2048, 1024
H, E = 16, 64
LN_EPS = 1e-5
SCALE = 8.0  # sqrt(E) * TEMP

N_CORES = 8
HL = H // 2          # heads per core
ST = S // 128        # 16 s-tiles of 128
KT = D // 128        # 8 contraction tiles for D
NP_ = HL // 2        # 4 head pairs per core
NB = S // 512        # 4 s-blocks of 512
TT = S // 128        # 16 t-tiles of 128

F32 = mybir.dt.float32
BF = mybir.dt.bfloat16

_NC_CACHE = None


def _emit(nc, aps):
    x_ap = aps["x"]
    xr_ap = aps["xr"]
    wq_ap, wk_ap, wv_ap, wo_ap = aps["wq"], aps["wk"], aps["wv"], aps["wo"]
    bq_ap, bk_ap, bv_ap = aps["bq"], aps["bk"], aps["bv"]
    out_ap = aps["out"]

    tc = aps["tc"]
    import contextlib

    ctx = contextlib.ExitStack()
    with ctx:
        const = ctx.enter_context(tc.tile_pool(name="const", bufs=1))
        big = ctx.enter_context(tc.tile_pool(name="big", bufs=1))
        xin = ctx.enter_context(tc.tile_pool(name="xin", bufs=4))
        stat = ctx.enter_context(tc.tile_pool(name="stat", bufs=8))
        xnp = ctx.enter_context(tc.tile_pool(name="xnp", bufs=6))
        ptp = ctx.enter_context(tc.tile_pool(name="ptp", bufs=3))
        denp = ctx.enter_context(tc.tile_pool(name="denp", bufs=2))
        rdenp = ctx.enter_context(tc.tile_pool(name="rdenp", bufs=2))
        xrp = ctx.enter_context(tc.tile_pool(name="xrp", bufs=3))
        outp = ctx.enter_context(tc.tile_pool(name="outp", bufs=4))
        psA = ctx.enter_context(tc.tile_pool(name="psA", bufs=2, space="PSUM"))
        psS = ctx.enter_context(tc.tile_pool(name="psS", bufs=2, space="PSUM"))
        psB = ctx.enter_context(tc.tile_pool(name="psB", bufs=2, space="PSUM"))

        # ---- constants / weights resident in SBUF ----
        wq_sb = const.tile([128, KT, 512], BF, tag="wq")
        wk_sb = const.tile([128, KT, 512], BF, tag="wk")
        wv_sb = const.tile([128, KT, 512], BF, tag="wv")
        for k in range(KT):
            nc.gpsimd.dma_start(out=wq_sb[:, k, :], in_=wq_ap[k])
            nc.gpsimd.dma_start(out=wk_sb[:, k, :], in_=wk_ap[k])
            nc.gpsimd.dma_start(out=wv_sb[:, k, :], in_=wv_ap[k])
        wo_sb = const.tile([128, 4, 1024], BF, tag="wo")
        for k in range(4):
            nc.gpsimd.dma_start(out=wo_sb[:, k, :], in_=wo_ap[k])
        bq_sb = const.tile([128, NP_], F32, tag="bq")
        bk_sb = const.tile([128, NP_], F32, tag="bk")
        nc.gpsimd.dma_start(out=bq_sb, in_=bq_ap)
        nc.gpsimd.dma_start(out=bk_sb, in_=bk_ap)
        bv_sb = const.tile([128, 512], F32, tag="bv")
        bv_bcast = bass.AP(
            tensor=bv_ap.tensor,
            offset=bv_ap.offset,
            ap=[[0, 128], [1, 512]],
        )
        nc.gpsimd.dma_start(out=bv_sb, in_=bv_bcast)
        ident = const.tile([128, 128], BF, tag="id")
        make_identity(nc, ident)
        ones64 = const.tile([1, 64], F32, tag="o64")
        nc.vector.memset(ones64, 1.0)
        onescol = const.tile([128, 1], BF, tag="oc")
        nc.vector.memset(onescol, 1.0)
        eps_t = const.tile([128, 1], F32, tag="eps")
        nc.vector.memset(eps_t, LN_EPS)

        xnT = big.tile([128, KT, S], BF, tag="xnT")     # [d, s] transposed LN(x)
        qT = big.tile([128, NP_, S], BF, tag="qT")      # [(pairhead,e), s]
        kT_ = big.tile([128, NP_, S], BF, tag="kT")
        v_sb = big.tile([128, TT, 512], BF, tag="v")    # [t, (h,e)]
        hT = big.tile([128, 4, S], BF, tag="hT")        # [(h,e), s] attn output

        # ---- projection / LN / out-proj work units (emitted on demand) ----
        def emit_qk_proj(kind, p, n):
            w_sb, b_sb, dst = (
                (wq_sb, bq_sb, qT) if kind == "q" else (wk_sb, bk_sb, kT_)
            )
            ps = psA.tile([128, 512], F32, tag="ps", name=f"proj_{kind}_{p}_{n}")
            for k in range(KT):
                nc.tensor.matmul(
                    ps, lhsT=w_sb[:, k, ts(p, 128)], rhs=xnT[:, k, ts(n, 512)],
                    start=(k == 0), stop=(k == KT - 1),
                )
            nc.vector.tensor_scalar_add(
                out=dst[:, p, ts(n, 512)], in0=ps, scalar1=b_sb[:, p:p + 1]
            )

        def emit_v_proj(t):
            ps = psA.tile([128, 512], F32, tag="ps", name=f"proj_v_{t}")
            for k in range(KT):
                nc.tensor.matmul(
                    ps, lhsT=xnT[:, k, ts(t, 128)], rhs=wv_sb[:, k, :],
                    start=(k == 0), stop=(k == KT - 1),
                )
            nc.vector.tensor_add(out=v_sb[:, t, :], in0=ps, in1=bv_sb)

        def emit_out_tile(i):
            xr_t = xrp.tile([128, D], F32, tag="xr", name=f"xr_{i}")
            nc.sync.dma_start(out=xr_t, in_=xr_ap[ts(i, 128), :])
            for c in range(2):
                ps_o = psA.tile([128, 512], F32, tag="ps", name=f"pso_{i}_{c}")
                for k in range(4):
                    nc.tensor.matmul(
                        ps_o, lhsT=hT[:, k, ts(i, 128)], rhs=wo_sb[:, k, ts(c, 512)],
                        start=(k == 0), stop=(k == 3),
                    )
                osb = outp.tile([128, 512], F32, tag="ob", name=f"ob_{i}_{c}")
                nc.vector.tensor_add(out=osb, in0=ps_o, in1=xr_t[:, ts(c, 512)])
                nc.sync.dma_start(out=out_ap[ts(i, 128), ts(c, 512)], in_=osb)

        # ---- phase 1: LayerNorm + transpose; pair-0 projections inline ----
        for i in range(ST):
            x_t = xin.tile([128, D], F32, tag="x")
            nc.sync.dma_start(out=x_t, in_=x_ap[ts(i, 128), :])
            stats = stat.tile([128, 2, 6], F32, tag="st")
            for sg in range(2):
                nc.vector.bn_stats(out=stats[:, sg, :], in_=x_t[:, ts(sg, 512)])
            mv = stat.tile([128, 2], F32, tag="mv")
            nc.vector.bn_aggr(out=mv, in_=stats)
            std = stat.tile([128, 1], F32, tag="sd")
            nc.scalar.activation(
                out=std, in_=mv[:, 1:2],
                func=mybir.ActivationFunctionType.Sqrt, bias=eps_t,
            )
            istd = stat.tile([128, 1], F32, tag="is")
            nc.vector.reciprocal(out=istd, in_=std)
            xn_t = xnp.tile([128, D], BF, tag="xn")
            nc.vector.tensor_scalar(
                out=xn_t, in0=x_t,
                scalar1=mv[:, 0:1], scalar2=istd,
                op0=mybir.AluOpType.subtract, op1=mybir.AluOpType.mult,
            )
            for k in range(KT):
                ps_tr = psA.tile([128, 128], BF, tag="ps")
                nc.tensor.transpose(out=ps_tr, in_=xn_t[:, ts(k, 128)], identity=ident)
                nc.vector.tensor_copy(out=xnT[:, k, ts(i, 128)], in_=ps_tr)
            # prologue compute that only needs LN tiles <= i
            emit_v_proj(i)
            if i % 4 == 3:
                emit_qk_proj("k", 0, i // 4)
                emit_qk_proj("q", 0, i // 4)

        # later pairs' projections stream into the ACT-bound attention
        # window, in dependency order (consumed one unit per designated slot)
        work_queue = [
            (kind, p, n)
            for p in range(1, NP_)
            for n in range(NB)
            for kind in ("k", "q")
        ]

        def pop_work():
            if not work_queue:
                return
            u = work_queue.pop(0)
            if u[0] == "v":
                emit_v_proj(u[1])
            else:
                emit_qk_proj(*u)

        # ---- phase 3: attention (scores^T -> exp -> PV), per head-pair/s-block --
        def emit_qk(p, n, t):
            s12 = psS.tile([128, 2, 512], F32, tag="ps2", name=f"s12_{p}_{n}_{t}")
            nc.tensor.matmul(
                s12[:, 0, :], lhsT=kT_[0:64, p, ts(t, 128)], rhs=qT[0:64, p, ts(n, 512)],
                start=True, stop=True, tile_position=(0, 0),
            )
            nc.tensor.matmul(
                s12[:, 1, :], lhsT=kT_[64:128, p, ts(t, 128)], rhs=qT[64:128, p, ts(n, 512)],
                start=True, stop=True, tile_position=(64, 0),
            )
            return s12

        def emit_epilogue(p, n, den, pvps):
            # denominators: fp32 partition-reduce via ones matmul
            ps_d1 = psA.tile([1, 512], F32, tag="ps", name=f"psd1_{p}_{n}")
            ps_d2 = psA.tile([1, 512], F32, tag="ps", name=f"psd2_{p}_{n}")
            nc.tensor.matmul(ps_d1, lhsT=onescol, rhs=den[:, 0, :],
                             start=True, stop=True, skip_group_check=True)
            nc.tensor.matmul(ps_d2, lhsT=onescol, rhs=den[:, 1, :],
                             start=True, stop=True, skip_group_check=True)
            recip = rdenp.tile([1, 1024], F32, tag="rd", name=f"rd_{p}_{n}")
            nc.vector.reciprocal(out=recip[0:1, 0:512], in_=ps_d1)
            nc.vector.reciprocal(out=recip[0:1, 512:1024], in_=ps_d2)
            ps_db = psA.tile([128, 512], F32, tag="ps", name=f"psdb_{p}_{n}")
            nc.tensor.matmul(
                ps_db[0:64, :], lhsT=ones64, rhs=recip[0:1, 0:512],
                start=True, stop=True, tile_position=(0, 0), skip_group_check=True,
            )
            nc.tensor.matmul(
                ps_db[64:128, :], lhsT=ones64, rhs=recip[0:1, 512:1024],
                start=True, stop=True, tile_position=(0, 64), skip_group_check=True,
            )
            db_sb = rdenp.tile([128, 512], F32, tag="db", name=f"db_{p}_{n}")
            nc.vector.tensor_copy(out=db_sb, in_=ps_db)
            nc.vector.tensor_mul(out=hT[:, p, ts(n, 512)], in0=pvps, in1=db_sb)

        pending = None
        s12_next = emit_qk(0, 0, 0)
        for p in range(NP_):
            for n in range(NB):
                first_block = (p, n) == (0, 0)
                den = denp.tile([128, 2, 512], BF, tag="den", name=f"den_{p}_{n}")
                pvps = psB.tile([128, 512], F32, tag="pv", name=f"pv_{p}_{n}")
                for t in range(TT):
                    s12 = s12_next
                    # next scores tile ahead of this tile's PV so PE stays busy
                    # while ACT runs exp(t)
                    if t < TT - 1:
                        s12_next = emit_qk(p, n, t + 1)
                    elif (p, n) != (NP_ - 1, NB - 1):
                        np_, nn = (p, n + 1) if n < NB - 1 else (p + 1, 0)
                        s12_next = emit_qk(np_, nn, 0)
                    pt = ptp.tile([128, 2, 512], BF, tag="pt", name=f"pt_{p}_{n}_{t}")
                    nc.scalar.activation(out=pt, in_=s12, func=mybir.ActivationFunctionType.Exp)
                    if t == 0:
                        nc.vector.tensor_copy(out=den, in_=pt)
                    else:
                        nc.vector.tensor_add(out=den, in0=den, in1=pt)
                    nc.tensor.matmul(
                        pvps[0:64, :], lhsT=v_sb[:, t, p * 128:p * 128 + 64], rhs=pt[:, 0, :],
                        start=(t == 0), stop=(t == TT - 1), tile_position=(0, 0),
                        skip_group_check=True,
                    )
                    nc.tensor.matmul(
                        pvps[64:128, :], lhsT=v_sb[:, t, p * 128 + 64:p * 128 + 128], rhs=pt[:, 1, :],
                        start=(t == 0), stop=(t == TT - 1), tile_position=(0, 64),
                        skip_group_check=True,
                    )
                    if t == 2 and pending is not None:
                        emit_epilogue(*pending)
                        pending = None
                    # stream deferred projections into the ACT-bound window
                    if t in (5, 9, 13):
                        pop_work()
                    # out-projection for earlier s-tiles once all pairs done
                    if p == NP_ - 1 and n >= 1 and t in (2, 6, 10, 14):
                        emit_out_tile(4 * (n - 1) + (t - 2) // 4)
                pending = (p, n, den, pvps)
        emit_epilogue(*pending)
        # remaining out-projection tiles
        for i in range(4 * (NB - 1), ST):
            emit_out_tile(i)


def build():
    nc = bacc.Bacc("TRN2", target_bir_lowering=False, debug=False, num_devices=N_CORES)
    aps = {
        "x": nc.dram_tensor("x", [S, D], F32, kind="ExternalInput").ap(),
        "xr": nc.dram_tensor("xr", [S, D], F32, kind="ExternalInput").ap(),
        "wq": nc.dram_tensor("wq", [KT, 128, 512], BF, kind="ExternalInput").ap(),
        "wk": nc.dram_tensor("wk", [KT, 128, 512], BF, kind="ExternalInput").ap(),
        "wv": nc.dram_tensor("wv", [KT, 128, 512], BF, kind="ExternalInput").ap(),
        "wo": nc.dram_tensor("wo", [4, 128, 1024], BF, kind="ExternalInput").ap(),
        "bq": nc.dram_tensor("bq", [128, NP_], F32, kind="ExternalInput").ap(),
        "bk": nc.dram_tensor("bk", [128, NP_], F32, kind="ExternalInput").ap(),
        "bv": nc.dram_tensor("bv", [512], F32, kind="ExternalInput").ap(),
        "out": nc.dram_tensor("out", [S, D], F32, kind="ExternalOutput").ap(),
    }
    with tile.TileContext(nc) as tc:
        aps["tc"] = tc
        _emit(nc, aps)
    nc.compile()
    return nc


def prep_core_inputs(x, Wq, bq, Wk, bk, Wv, bv, Wo, bo, ln_gamma, ln_beta):
    """Host-side sharding: returns list of 8 in_maps (numpy arrays)."""
    x = np.asarray(x, np.float32)
    Wq, bq = np.asarray(Wq, np.float32), np.asarray(bq, np.float32)
    Wk, bk = np.asarray(Wk, np.float32), np.asarray(bk, np.float32)
    Wv, bv = np.asarray(Wv, np.float32), np.asarray(bv, np.float32)
    Wo, bo = np.asarray(Wo, np.float32), np.asarray(bo, np.float32)
    gamma, beta = np.asarray(ln_gamma, np.float32), np.asarray(ln_beta, np.float32)

    Wq_eff = Wq * gamma[None, None, :] / SCALE
    bq_eff = (bq + Wq @ beta) / SCALE
    Wk_eff = Wk * gamma[None, None, :]
    bk_eff = bk + Wk @ beta
    Wv_eff = Wv * gamma[None, None, :]
    bv_eff = bv + Wv @ beta

    def wq_layout(w):  # [8, 64, 1024] -> [KT, 128, 512]
        # w[h, e, kt*128+dd] -> out[kt, dd, h*64+e]
        return np.ascontiguousarray(
            w.reshape(HL * E, KT, 128).transpose(1, 2, 0)
        ).astype(BF_NP)

    def b_layout(b):  # [8, 64] -> [128, 4]: out[(hh*64+e), p] = b[2p+hh, e]
        return np.ascontiguousarray(
            b.reshape(NP_, 2 * E).T
        ).astype(np.float32)

    in_maps = []
    for c in range(N_CORES):
        bidx, g = c // 2, c % 2
        hs = slice(g * HL, (g + 1) * HL)
        wo_loc = Wo[:, g * 512:(g + 1) * 512]  # [1024, 512]
        wo_dev = np.ascontiguousarray(
            wo_loc.T.reshape(4, 128, 1024)
        ).astype(BF_NP)
        in_maps.append({
            "x": x[bidx],
            "xr": 0.5 * (x[bidx] + bo[None, :]),
            "wq": wq_layout(Wq_eff[hs]),
            "wk": wq_layout(Wk_eff[hs]),
            "wv": wq_layout(Wv_eff[hs]),
            "wo": wo_dev,
            "bq": b_layout(bq_eff[hs]),
            "bk": b_layout(bk_eff[hs]),
            "bv": bv_eff[hs].reshape(512).astype(np.float32),
            "out": np.zeros((S, D), np.float32),
        })
    return in_maps


def kernel(x, Wq, bq, Wk, bk, Wv, bv, Wo, bo, ln_gamma, ln_beta):
    global _NC_CACHE
    if _NC_CACHE is None:
        _NC_CACHE = build()
    nc = _NC_CACHE
    in_maps = prep_core_inputs(x, Wq, bq, Wk, bk, Wv, bv, Wo, bo, ln_gamma, ln_beta)
    for m in in_maps:
        m.pop("out")
    res = bass_utils.run_bass_kernel_spmd(nc, in_maps, core_ids=list(range(N_CORES)))
    out = np.empty((B, S, D), np.float32)
    for bidx in range(B):
        out[bidx] = res.results[2 * bidx]["out"] + res.results[2 * bidx + 1]["out"]
    return out


# revision 43
# speedup vs baseline: 13437.8797x; 13437.8797x over previous
"""Multi-head attention block (pre-LN, residual) on 8 Trainium2 NeuronCores.

Sharding: (batch x head-group) grid. Core c handles batch b = c//2 and head
group g = c%2 (8 of 16 heads). Per core: LN(x_b) -> per-head QKV projections
-> softmax attention (no max-subtraction; scores are O(10)) -> out-projection
against the local 512-wide slice of Wo, + 0.5*(x+bo) residual. Host sums the
two partial outputs per batch (the pair all-reduce) and stacks batches.

LayerNorm gamma/beta are folded into the QKV weights/biases on the host
(exact: projections are linear in xn). The 1/sqrt(E) score scale is folded
into Wq. Matmul operands are bf16 with fp32 PSUM accumulation; LN statistics,
softmax denominators and the residual path stay fp32.
"""

import numpy as np
import ml_dtypes

import concourse.bass as bass
import concourse.mybir as mybir
import concourse.tile as tile
from concourse import bacc
from concourse import bass_utils
from concourse.bass import ts
from concourse.masks import make_identity

BF_NP = ml_dtypes.bfloat16

B, S, D = 4, 2048, 1024
H, E = 16, 64
LN_EPS = 1e-5
SCALE = 8.0  # sqrt(E) * TEMP

N_CORES = 8
HL = H // 2          # heads per core
ST = S // 128        # 16 s-tiles of 128
KT = D // 128        # 8 contraction tiles for D
NP_ = HL // 2        # 4 head pairs per core
NB = S // 512        # 4 s-blocks of 512
TT = S // 128        # 16 t-tiles of 128

F32 = mybir.dt.float32
BF = mybir.dt.bfloat16

_NC_CACHE = None


def _emit(nc, aps):
    x_ap = aps["x"]
    xr_ap = aps["xr"]
    wq_ap, wk_ap, wv_ap, wo_ap = aps["wq"], aps["wk"], aps["wv"], aps["wo"]
    bq_ap, bk_ap, bv_ap = aps["bq"], aps["bk"], aps["bv"]
    out_ap = aps["out"]

    tc = aps["tc"]
    import contextlib

    ctx = contextlib.ExitStack()
    with ctx:
        const = ctx.enter_context(tc.tile_pool(name="const", bufs=1))
        big = ctx.enter_context(tc.tile_pool(name="big", bufs=1))
        xin = ctx.enter_context(tc.tile_pool(name="xin", bufs=4))
        stat = ctx.enter_context(tc.tile_pool(name="stat", bufs=8))
        xnp = ctx.enter_context(tc.tile_pool(name="xnp", bufs=6))
        ptp = ctx.enter_context(tc.tile_pool(name="ptp", bufs=3))
        denp = ctx.enter_context(tc.tile_pool(name="denp", bufs=2))
        rdenp = ctx.enter_context(tc.tile_pool(name="rdenp", bufs=2))
        xrp = ctx.enter_context(tc.tile_pool(name="xrp", bufs=3))
        outp = ctx.enter_context(tc.tile_pool(name="outp", bufs=4))
        psA = ctx.enter_context(tc.tile_pool(name="psA", bufs=2, space="PSUM"))
        psS = ctx.enter_context(tc.tile_pool(name="psS", bufs=2, space="PSUM"))
        psB = ctx.enter_context(tc.tile_pool(name="psB", bufs=2, space="PSUM"))

        # ---- constants / weights resident in SBUF ----
        wq_sb = const.tile([128, KT, 512], BF, tag="wq")
        wk_sb = const.tile([128, KT, 512], BF, tag="wk")
        wv_sb = const.tile([128, KT, 512], BF, tag="wv")
        for k in range(KT):
            nc.gpsimd.dma_start(out=wq_sb[:, k, :], in_=wq_ap[k])
            nc.gpsimd.dma_start(out=wk_sb[:, k, :], in_=wk_ap[k])
            nc.gpsimd.dma_start(out=wv_sb[:, k, :], in_=wv_ap[k])
        wo_sb = const.tile([128, 4, 1024], BF, tag="wo")
        for k in range(4):
            nc.gpsimd.dma_start(out=wo_sb[:, k, :], in_=wo_ap[k])
        bq_sb = const.tile([128, NP_], F32, tag="bq")
        bk_sb = const.tile([128, NP_], F32, tag="bk")
        nc.gpsimd.dma_start(out=bq_sb, in_=bq_ap)
        nc.gpsimd.dma_start(out=bk_sb, in_=bk_ap)
        bv_sb = const.tile([128, 512], F32, tag="bv")
        bv_bcast = bass.AP(
            tensor=bv_ap.tensor,
            offset=bv_ap.offset,
            ap=[[0, 128], [1, 512]],
        )
        nc.gpsimd.dma_start(out=bv_sb, in_=bv_bcast)
        ident = const.tile([128, 128], BF, tag="id")
        make_identity(nc, ident)
        ones64 = const.tile([1, 64], F32, tag="o64")
        nc.vector.memset(ones64, 1.0)
        onescol = const.tile([128, 1], BF, tag="oc")
        nc.vector.memset(onescol, 1.0)
        eps_t = const.tile([128, 1], F32, tag="eps")
        nc.vector.memset(eps_t, LN_EPS)

        xnT = big.tile([128, KT, S], BF, tag="xnT")     # [d, s] transposed LN(x)
        qT = big.tile([128, NP_, S], BF, tag="qT")      # [(pairhead,e), s]
        kT_ = big.tile([128, NP_, S], BF, tag="kT")
        v_sb = big.tile([128, TT, 512], BF, tag="v")    # [t, (h,e)]
        hT = big.tile([128, 4, S], BF, tag="hT")        # [(h,e), s] attn output

        # ---- projection / LN / out-proj work units (emitted on demand) ----
        def emit_qk_proj(kind, p, n):
            w_sb, b_sb, dst = (
                (wq_sb, bq_sb, qT) if kind == "q" else (wk_sb, bk_sb, kT_)
            )
            ps = psA.tile([128, 512], F32, tag="ps", name=f"proj_{kind}_{p}_{n}")
            for k in range(KT):
                nc.tensor.matmul(
                    ps, lhsT=w_sb[:, k, ts(p, 128)], rhs=xnT[:, k, ts(n, 512)],
                    start=(k == 0), stop=(k == KT - 1),
                )
            nc.vector.tensor_scalar_add(
                out=dst[:, p, ts(n, 512)], in0=ps, scalar1=b_sb[:, p:p + 1]
            )

        def emit_v_proj(t):
            ps = psA.tile([128, 512], F32, tag="ps", name=f"proj_v_{t}")
            for k in range(KT):
                nc.tensor.matmul(
                    ps, lhsT=xnT[:, k, ts(t, 128)], rhs=wv_sb[:, k, :],
                    start=(k == 0), stop=(k == KT - 1),
                )
            nc.vector.tensor_add(out=v_sb[:, t, :], in0=ps, in1=bv_sb)

        def emit_out_tile(i):
            xr_t = xrp.tile([128, D], F32, tag="xr", name=f"xr_{i}")
            nc.sync.dma_start(out=xr_t, in_=xr_ap[ts(i, 128), :])
            for c in range(2):
                ps_o = psA.tile([128, 512], F32, tag="ps", name=f"pso_{i}_{c}")
                for k in range(4):
                    nc.tensor.matmul(
                        ps_o, lhsT=hT[:, k, ts(i, 128)], rhs=wo_sb[:, k, ts(c, 512)],
                        start=(k == 0), stop=(k == 3),
                    )
                osb = outp.tile([128, 512], F32, tag="ob", name=f"ob_{i}_{c}")
                nc.vector.tensor_add(out=osb, in0=ps_o, in1=xr_t[:, ts(c, 512)])
                nc.sync.dma_start(out=out_ap[ts(i, 128), ts(c, 512)], in_=osb)

        # ---- phase 1: LayerNorm + transpose; pair-0 projections inline ----
        for i in range(ST):
            x_t = xin.tile([128, D], F32, tag="x")
            nc.sync.dma_start(out=x_t, in_=x_ap[ts(i, 128), :])
            stats = stat.tile([128, 2, 6], F32, tag="st")
            for sg in range(2):
                nc.vector.bn_stats(out=stats[:, sg, :], in_=x_t[:, ts(sg, 512)])
            mv = stat.tile([128, 2], F32, tag="mv")
            nc.vector.bn_aggr(out=mv, in_=stats)
            std = stat.tile([128, 1], F32, tag="sd")
            nc.scalar.activation(
                out=std, in_=mv[:, 1:2],
                func=mybir.ActivationFunctionType.Sqrt, bias=eps_t,
            )
            istd = stat.tile([128, 1], F32, tag="is")
            nc.vector.reciprocal(out=istd, in_=std)
            xn_t = xnp.tile([128, D], BF, tag="xn")
            nc.vector.tensor_scalar(
                out=xn_t, in0=x_t,
                scalar1=mv[:, 0:1], scalar2=istd,
                op0=mybir.AluOpType.subtract, op1=mybir.AluOpType.mult,
            )
            for k in range(KT):
                ps_tr = psA.tile([128, 128], BF, tag="ps")
                nc.tensor.transpose(out=ps_tr, in_=xn_t[:, ts(k, 128)], identity=ident)
                nc.vector.tensor_copy(out=xnT[:, k, ts(i, 128)], in_=ps_tr)
            # prologue compute that only needs LN tiles <= i
            emit_v_proj(i)
            if i % 4 == 3:
                emit_qk_proj("k", 0, i // 4)
                emit_qk_proj("q", 0, i // 4)

        # later pairs' projections stream into the ACT-bound attention
        # window, in dependency order (consumed one unit per designated slot)
        work_queue = [
            (kind, p, n)
            for p in range(1, NP_)
            for n in range(NB)
            for kind in ("k", "q")
        ]

        def pop_work():
            if not work_queue:
                return
            u = work_queue.pop(0)
            if u[0] == "v":
                emit_v_proj(u[1])
            else:
                emit_qk_proj(*u)

        # ---- phase 3: attention (scores^T -> exp -> PV), per head-pair/s-block --
        def emit_qk(p, n, t):
            s12 = psS.tile([128, 2, 512], F32, tag="ps2", name=f"s12_{p}_{n}_{t}")
            nc.tensor.matmul(
                s12[:, 0, :], lhsT=kT_[0:64, p, ts(t, 128)], rhs=qT[0:64, p, ts(n, 512)],
                start=True, stop=True, tile_position=(0, 0),
            )
            nc.tensor.matmul(
                s12[:, 1, :], lhsT=kT_[64:128, p, ts(t, 128)], rhs=qT[64:128, p, ts(n, 512)],
                start=True, stop=True, tile_position=(64, 0),
            )
            return s12

        def emit_epilogue(p, n, den, pvps):
            # denominators: fp32 partition-reduce via ones matmul
            ps_d1 = psA.tile([1, 512], F32, tag="ps", name=f"psd1_{p}_{n}")
            ps_d2 = psA.tile([1, 512], F32, tag="ps", name=f"psd2_{p}_{n}")
            nc.tensor.matmul(ps_d1, lhsT=onescol, rhs=den[:, 0, :],
                             start=True, stop=True, skip_group_check=True)
            nc.tensor.matmul(ps_d2, lhsT=onescol, rhs=den[:, 1, :],
                             start=True, stop=True, skip_group_check=True)
            recip = rdenp.tile([1, 1024], F32, tag="rd", name=f"rd_{p}_{n}")
            nc.vector.reciprocal(out=recip[0:1, 0:512], in_=ps_d1)
            nc.vector.reciprocal(out=recip[0:1, 512:1024], in_=ps_d2)
            ps_db = psA.tile([128, 512], F32, tag="ps", name=f"psdb_{p}_{n}")
            nc.tensor.matmul(
                ps_db[0:64, :], lhsT=ones64, rhs=recip[0:1, 0:512],
                start=True, stop=True, tile_position=(0, 0), skip_group_check=True,
            )
            nc.tensor.matmul(
                ps_db[64:128, :], lhsT=ones64, rhs=recip[0:1, 512:1024],
                start=True, stop=True, tile_position=(0, 64), skip_group_check=True,
            )
            db_sb = rdenp.tile([128, 512], F32, tag="db", name=f"db_{p}_{n}")
            nc.vector.tensor_copy(out=db_sb, in_=ps_db)
            nc.vector.tensor_mul(out=hT[:, p, ts(n, 512)], in0=pvps, in1=db_sb)

        pending = None
        s12_next = emit_qk(0, 0, 0)
        for p in range(NP_):
            for n in range(NB):
                first_block = (p, n) == (0, 0)
                den = denp.tile([128, 2, 512], BF, tag="den", name=f"den_{p}_{n}")
                pvps = psB.tile([128, 512], F32, tag="pv", name=f"pv_{p}_{n}")
                for t in range(TT):
                    s12 = s12_next
                    # next scores tile ahead of this tile's PV so PE stays busy
                    # while ACT runs exp(t)
                    if t < TT - 1:
                        s12_next = emit_qk(p, n, t + 1)
                    elif (p, n) != (NP_ - 1, NB - 1):
                        np_, nn = (p, n + 1) if n < NB - 1 else (p + 1, 0)
                        s12_next = emit_qk(np_, nn, 0)
                    pt = ptp.tile([128, 2, 512], BF, tag="pt", name=f"pt_{p}_{n}_{t}")
                    nc.scalar.activation(out=pt, in_=s12, func=mybir.ActivationFunctionType.Exp)
                    if t == 0:
                        nc.vector.tensor_copy(out=den, in_=pt)
                    else:
                        nc.vector.tensor_add(out=den, in0=den, in1=pt)
                    nc.tensor.matmul(
                        pvps[0:64, :], lhsT=v_sb[:, t, p * 128:p * 128 + 64], rhs=pt[:, 0, :],
                        start=(t == 0), stop=(t == TT - 1), tile_position=(0, 0),
                        skip_group_check=True,
                    )
                    nc.tensor.matmul(
                        pvps[64:128, :], lhsT=v_sb[:, t, p * 128 + 64:p * 128 + 128], rhs=pt[:, 1, :],
                        start=(t == 0), stop=(t == TT - 1), tile_position=(0, 64),
                        skip_group_check=True,
                    )
                    if t == 2 and pending is not None:
                        emit_epilogue(*pending)
                        pending = None
                    # stream deferred projections into the ACT-bound window
                    if t in (5, 9, 13):
                        pop_work()
                    # out-projection for earlier s-tiles once all pairs done
                    if p == NP_ - 1 and n >= 1 and t in (2, 6, 10, 14):
                        emit_out_tile(4 * (n - 1) + (t - 2) // 4)
                pending = (p, n, den, pvps)
        emit_epilogue(*pending)
        # remaining out-projection tiles
        for i in range(4 * (NB - 1), ST):
            emit_out_tile(i)


def build():
    nc = bacc.Bacc("TRN2", target_bir_lowering=False, debug=False, num_devices=N_CORES)
    aps = {
        "x": nc.dram_tensor("x", [S, D], F32, kind="ExternalInput").ap(),
        "xr": nc.dram_tensor("xr", [S, D], F32, kind="ExternalInput").ap(),
        "wq": nc.dram_tensor("wq", [KT, 128, 512], BF, kind="ExternalInput").ap(),
        "wk": nc.dram_tensor("wk", [KT, 128, 512], BF, kind="ExternalInput").ap(),
        "wv": nc.dram_tensor("wv", [KT, 128, 512], BF, kind="ExternalInput").ap(),
        "wo": nc.dram_tensor("wo", [4, 128, 1024], BF, kind="ExternalInput").ap(),
        "bq": nc.dram_tensor("bq", [128, NP_], F32, kind="ExternalInput").ap(),
        "bk": nc.dram_tensor("bk", [128, NP_], F32, kind="ExternalInput").ap(),
        "bv": nc.dram_tensor("bv", [512], F32, kind="ExternalInput").ap(),
        "out": nc.dram_tensor("out", [S, D], F32, kind="ExternalOutput").ap(),
    }
    with tile.TileContext(nc) as tc:
        aps["tc"] = tc
        _emit(nc, aps)
    nc.compile()
    return nc


def prep_core_inputs(x, Wq, bq, Wk, bk, Wv, bv, Wo, bo, ln_gamma, ln_beta):
    """Host-side sharding: returns list of 8 in_maps (numpy arrays)."""
    x = np.asarray(x, np.float32)
    Wq, bq = np.asarray(Wq, np.float32), np.asarray(bq, np.float32)
    Wk, bk = np.asarray(Wk, np.float32), np.asarray(bk, np.float32)
    Wv, bv = np.asarray(Wv, np.float32), np.asarray(bv, np.float32)
    Wo, bo = np.asarray(Wo, np.float32), np.asarray(bo, np.float32)
    gamma, beta = np.asarray(ln_gamma, np.float32), np.asarray(ln_beta, np.float32)

    Wq_eff = Wq * gamma[None, None, :] / SCALE
    bq_eff = (bq + Wq @ beta) / SCALE
    Wk_eff = Wk * gamma[None, None, :]
    bk_eff = bk + Wk @ beta
    Wv_eff = Wv * gamma[None, None, :]
    bv_eff = bv + Wv @ beta

    def wq_layout(w):  # [8, 64, 1024] -> [KT, 128, 512]
        # w[h, e, kt*128+dd] -> out[kt, dd, h*64+e]
        return np.ascontiguousarray(
            w.reshape(HL * E, KT, 128).transpose(1, 2, 0)
        ).astype(BF_NP)

    def b_layout(b):  # [8, 64] -> [128, 4]: out[(hh*64+e), p] = b[2p+hh, e]
        return np.ascontiguousarray(
            b.reshape(NP_, 2 * E).T
        ).astype(np.float32)

    in_maps = []
    for c in range(N_CORES):
        bidx, g = c // 2, c % 2
        hs = slice(g * HL, (g + 1) * HL)
        wo_loc = Wo[:, g * 512:(g + 1) * 512]  # [1024, 512]
        wo_dev = np.ascontiguousarray(
            wo_loc.T.reshape(4, 128, 1024)
        ).astype(BF_NP)
        in_maps.append({
            "x": x[bidx],
            "xr": 0.5 * (x[bidx] + bo[None, :]),
            "wq": wq_layout(Wq_eff[hs]),
            "wk": wq_layout(Wk_eff[hs]),
            "wv": wq_layout(Wv_eff[hs]),
            "wo": wo_dev,
            "bq": b_layout(bq_eff[hs]),
            "bk": b_layout(bk_eff[hs]),
            "bv": bv_eff[hs].reshape(512).astype(np.float32),
            "out": np.zeros((S, D), np.float32),
        })
    return in_maps


def kernel(x, Wq, bq, Wk, bk, Wv, bv, Wo, bo, ln_gamma, ln_beta):
    global _NC_CACHE
    if _NC_CACHE is None:
        _NC_CACHE = build()
    nc = _NC_CACHE
    in_maps = prep_core_inputs(x, Wq, bq, Wk, bk, Wv, bv, Wo, bo, ln_gamma, ln_beta)
    for m in in_maps:
        m.pop("out")
    res = bass_utils.run_bass_kernel_spmd(nc, in_maps, core_ids=list(range(N_CORES)))
    out = np.empty((B, S, D), np.float32)
    for bidx in range(B):
        out[bidx] = res.results[2 * bidx]["out"] + res.results[2 * bidx + 1]["out"]
    return out
